# revision 1
# baseline (speedup 1.0000x reference)
"""CBOW negative-sampling loss kernel for Trainium2 (8 NeuronCores).

Problem (see reference):
    context_embeds = in_W[context].mean(axis=1)          # [B, D]
    true_embeds    = out_W[center.squeeze(1)]            # [B, D]
    pos_loss = softplus(-sum(context_embeds*true_embeds, -1)).mean()
    neg_embeds = out_W[neg_context]                      # [B, K, D]
    neg_loss = softplus(einsum('bkd,bd->bk', ...)).sum(-1).mean()
    out = pos_loss + neg_loss                            # scalar

Strategy: data-parallel over batch across 8 cores (2048 rows each);
embedding tables replicated per core.  Each core gathers its rows with
SWDGE indirect DMA (one 512B descriptor per embedding row), computes
dot products + softplus on DVE/ACT, and reduces to one partial-sum
scalar.  Host sums the 8 partials and divides by B.

Row layout per core: batch row b = chunk*128 + p lives on partition p,
chunk index c in the free dim (16 chunks of 128 rows).  Gathers are
issued per "super-chunk" of SC=4 chunks so DMA overlaps compute.

The walrus build in this container encodes at most ONE semaphore wait
per instruction ("Too many sync wait commands") and rejects the raw-ISA
InstTensorTensorReduce ("ISA wrong length"), so: waits are split onto
single-wait NoOps at Tile lowering time (PatchedTileContext below), and
dots use tensor_tensor + tensor_reduce instead.
"""

import numpy as np

VOCAB = 100000
DIM = 128
BATCH = 16384
CTX = 8
K_NEG = 10
N_CORES = 8
P = 128

B_CORE = BATCH // N_CORES          # 2048
N_CHUNKS = B_CORE // P             # 16
SC = 4                             # chunks per gather super-chunk
W_COLS = 1 + K_NEG                 # center + negatives share the out_W gather

_CACHE = {}


def _patched_tile_context():
    import concourse.mybir as mybir
    import concourse.tile as tile
    from concourse.vector_clock import ScopedClock

    class PatchedTileContext(tile.TileContext):
        """Split multi-wait sync_infos: this container's walrus codegen
        accepts only one semaphore wait (and update) per instruction."""

        def _add_instruction(self, inst):
            si = getattr(inst, "sync_info", None)
            if si is not None and len(si.on_wait) > 1:
                waits = list(si.on_wait)
                for w in waits[:-1]:
                    nop = mybir.InstNoOp(
                        name=f"I-{self.nc.next_id()}-waitsplit",
                        engine=inst.engine,
                        sync_info=mybir.SyncInfo(on_wait=[w], on_update=[]),
                        bass_nofuse=True,
                    )
                    super()._add_instruction(nop)
                inst.sync_info = mybir.SyncInfo(
                    on_wait=[waits[-1]], on_update=list(si.on_update)
                )
            super()._add_instruction(inst)

        def _drain_and_barrier(self, tick_clock, wait_clock):
            drain_inst = self.nc.sync.drain()
            wait_clock.add_sem_waits(
                drain_inst.ins, ScopedClock({None: tick_clock.global_clock})
            )
            si = drain_inst.ins.sync_info
            if si is not None and len(si.on_wait) > 1:
                waits = list(si.on_wait)
                ups = list(si.on_update)
                drain_inst.ins.sync_info = mybir.SyncInfo(
                    on_wait=waits[:1], on_update=[]
                )
                for i, w in enumerate(waits[1:]):
                    d2 = self.nc.sync.drain()
                    last = i == len(waits) - 2
                    d2.ins.sync_info = mybir.SyncInfo(
                        on_wait=[w], on_update=ups if last else []
                    )
            self.nc.all_engine_barrier()
            popped = self.nc._tile_sem_poison_stack.pop()
            assert popped is self._sem_poison
            self.nc.clear_and_free_semaphores(list(self.sems.allocated().values()))
            self.nc.all_engine_barrier()

    return PatchedTileContext


def build_bass(vocab=VOCAB, n_chunks=N_CHUNKS, sc=SC, gather_bufs=1):
    """Build the per-core Bass program.  Tables are gathered as bf16."""
    import concourse.bass as bass
    import concourse.mybir as mybir

    f32 = mybir.dt.float32
    bf16 = mybir.dt.bfloat16
    i32 = mybir.dt.int32
    n_sc = n_chunks // sc
    TileContext = _patched_tile_context()

    nc = bass.Bass()

    idx_d = nc.dram_tensor("idx_all", [P, n_chunks * (CTX + W_COLS)], i32, kind="ExternalInput")
    in_w_d = nc.dram_tensor("in_w", [vocab, DIM], bf16, kind="ExternalInput")
    out_w_d = nc.dram_tensor("out_w", [vocab, DIM], bf16, kind="ExternalInput")
    loss_d = nc.dram_tensor("loss", [P, 2], f32, kind="ExternalOutput")

    with TileContext(nc) as tc:
        with (
            nc.allow_low_precision(reason="bf16 dots are well within tolerance here"),
            tc.tile_pool(name="idx", bufs=1) as ipool,
            tc.tile_pool(name="gather", bufs=gather_bufs) as gpool,
            tc.tile_pool(name="work", bufs=3) as wpool,
            tc.tile_pool(name="accp", bufs=1) as apool,
        ):
            idx_all = ipool.tile([P, n_chunks * (CTX + W_COLS)], i32)
            nc.sync.dma_start(out=idx_all[:], in_=idx_d[:])
            ctx_idx = idx_all[:, :n_chunks * CTX]
            w_idx = idx_all[:, n_chunks * CTX:]

            acc = apool.tile([P, n_sc], f32)           # per-super-chunk row losses
            pos_acc = apool.tile([P, n_chunks], f32)   # raw pos dots per chunk

            # issue ALL gathers first so the Pool engine streams descriptors
            # back-to-back and the SDMA queue never starves
            sc_tiles = []
            for s in range(n_sc):
                ctx_g = gpool.tile([P, sc * CTX * DIM], bf16, tag=f"ctx_g{s}")
                w_g = gpool.tile([P, sc * W_COLS * DIM], bf16, tag=f"w_g{s}")
                nc.gpsimd.indirect_dma_start(
                    out=ctx_g[:],
                    out_offset=None,
                    in_=in_w_d[:],
                    in_offset=bass.IndirectOffsetOnAxis(
                        ap=ctx_idx[:, s * sc * CTX:(s + 1) * sc * CTX], axis=0
                    ),
                )
                nc.gpsimd.indirect_dma_start(
                    out=w_g[:],
                    out_offset=None,
                    in_=out_w_d[:],
                    in_offset=bass.IndirectOffsetOnAxis(
                        ap=w_idx[:, s * sc * W_COLS:(s + 1) * sc * W_COLS], axis=0
                    ),
                )
                sc_tiles.append((ctx_g, w_g))

            for s in range(n_sc):
                ctx_g, w_g = sc_tiles[s]
                # context sum over k (CTX gathered rows), whole super-chunk,
                # as a contiguous-inner tree of adds (DVE 2x bf16 mode; a
                # strided reduce-X runs at <1x and is ~3x slower)
                cv = ctx_g[:].rearrange("p (c k d) -> p c k d", c=sc, k=CTX)
                t1 = wpool.tile([P, sc * 4 * DIM], bf16, tag="t1")
                t1v = t1[:].rearrange("p (c k d) -> p c k d", c=sc, k=4)
                nc.vector.tensor_add(out=t1v, in0=cv[:, :, 0:4, :], in1=cv[:, :, 4:8, :])
                t2 = wpool.tile([P, sc * 2 * DIM], bf16, tag="t2")
                t2v = t2[:].rearrange("p (c k d) -> p c k d", c=sc, k=2)
                nc.vector.tensor_add(out=t2v, in0=t1v[:, :, 0:2, :], in1=t1v[:, :, 2:4, :])
                cs = wpool.tile([P, sc * DIM], bf16, tag="cs")
                csv = cs[:].rearrange("p (c o d) -> p c o d", c=sc, o=1)
                nc.vector.tensor_add(out=csv, in0=t2v[:, :, 0:1, :], in1=t2v[:, :, 1:2, :])

                # prod[p, c, t, d] = w_g[p, c, t, d] * cs[p, c, d]
                prod = wpool.tile([P, sc * W_COLS * DIM], bf16, tag="prod")
                nc.vector.tensor_mul(
                    out=prod[:],
                    in0=w_g[:],
                    in1=cs[:].rearrange("p (c o d) -> p c o d", c=sc, o=1).broadcast_to(
                        [P, sc, W_COLS, DIM]
                    ),
                )
                # fold d 128 -> 32 with adds (2x mode) before the 1x reduce
                pv = prod[:].rearrange("p (c t h d) -> p c t h d", c=sc, t=W_COLS, h=2)
                f1 = wpool.tile([P, sc * W_COLS * 64], bf16, tag="f1")
                f1v = f1[:].rearrange("p (c t h d) -> p c t h d", c=sc, t=W_COLS, h=2)
                nc.vector.tensor_add(
                    out=f1[:].rearrange("p (c t d) -> p c t d", c=sc, t=W_COLS),
                    in0=pv[:, :, :, 0, :], in1=pv[:, :, :, 1, :],
                )
                f2 = wpool.tile([P, sc * W_COLS * 32], bf16, tag="f2")
                nc.vector.tensor_add(
                    out=f2[:].rearrange("p (c t d) -> p c t d", c=sc, t=W_COLS),
                    in0=f1v[:, :, :, 0, :], in1=f1v[:, :, :, 1, :],
                )
                f2v = f2[:].rearrange("p (c t h d) -> p c t h d", c=sc, t=W_COLS, h=2)
                f3 = wpool.tile([P, sc * W_COLS * 16], bf16, tag="f3")
                nc.vector.tensor_add(
                    out=f3[:].rearrange("p (c t d) -> p c t d", c=sc, t=W_COLS),
                    in0=f2v[:, :, :, 0, :], in1=f2v[:, :, :, 1, :],
                )
                dots = wpool.tile([P, sc * W_COLS], f32, tag="dots")
                nc.vector.reduce_sum(
                    out=dots[:],
                    in_=f3[:].rearrange("p (c t d) -> p c t d", c=sc, t=W_COLS),
                    axis=mybir.AxisListType.X,
                )

                # softplus identity: softplus(-pos/8) = softplus(pos/8) - pos/8,
                # so apply softplus(x/8) to ALL 11 columns (contiguous ACT ops)
                # and subtract the pos dots at the end (host combines).
                es = wpool.tile([P, sc * W_COLS], f32, tag="es")
                sp = wpool.tile([P, sc * W_COLS], f32, tag="sp")
                nc.scalar.activation(
                    out=es[:], in_=dots[:],
                    func=mybir.ActivationFunctionType.Exp, scale=1.0 / CTX,
                )
                nc.scalar.activation(
                    out=sp[:], in_=es[:],
                    func=mybir.ActivationFunctionType.Ln, bias=1.0,
                    accum_out=acc[:, s:s + 1],
                )
                # stash the pos dots (t=0 column of each chunk) for correction
                nc.vector.tensor_copy(
                    out=pos_acc[:, s * sc:(s + 1) * sc],
                    in_=dots[:].rearrange("p (c t) -> p c t", t=W_COLS)[:, :, 0:1],
                )

            # partials [p, 0] = sum of softplus(x/8) terms, [p, 1] = sum of
            # raw pos dots; host sums partitions: (sum0 - sum1/8) / BATCH
            partials = apool.tile([P, 2], f32)
            nc.vector.reduce_sum(
                out=partials[:, 0:1], in_=acc[:], axis=mybir.AxisListType.X
            )
            nc.vector.reduce_sum(
                out=partials[:, 1:2], in_=pos_acc[:], axis=mybir.AxisListType.X
            )
            nc.sync.dma_start(out=loss_d[:], in_=partials[:])

    nc.finalize()
    return nc


def pack_indices(center, context, neg_context, n_chunks=N_CHUNKS):
    """Pack per-core indices into the SBUF layouts the kernel expects.

    ctx_idx [P, n_chunks*CTX]: [p, c*CTX + k] = context[c*128 + p, k]
    w_idx   [P, n_chunks*11]:  [p, c*11 + 0] = center row, +1.. = negatives
    """
    rows = n_chunks * P
    ctx_l, w_l = [], []
    for m in range(N_CORES):
        lo = m * rows
        ctx = np.ascontiguousarray(context[lo:lo + rows]).astype(np.int32)
        cen = np.ascontiguousarray(center[lo:lo + rows]).astype(np.int32)
        neg = np.ascontiguousarray(neg_context[lo:lo + rows]).astype(np.int32)
        ctx_p = ctx.reshape(n_chunks, P, CTX).transpose(1, 0, 2).reshape(P, n_chunks * CTX)
        w = np.concatenate([cen.reshape(rows, 1), neg.reshape(rows, K_NEG)], axis=1)
        w_p = w.reshape(n_chunks, P, W_COLS).transpose(1, 0, 2).reshape(P, n_chunks * W_COLS)
        ctx_l.append(np.ascontiguousarray(ctx_p))
        w_l.append(np.ascontiguousarray(w_p))
    return ctx_l, w_l


def kernel(center, context, neg_context, in_W, out_W):
    from concourse.bass_utils import run_bass_kernel_spmd

    if "nc" not in _CACHE:
        _CACHE["nc"] = build_bass()
    nc = _CACHE["nc"]

    import ml_dtypes

    ctx_l, w_l = pack_indices(np.asarray(center), np.asarray(context), np.asarray(neg_context))
    idx_l = [np.ascontiguousarray(np.concatenate([c, w], axis=1)) for c, w in zip(ctx_l, w_l)]
    in_w = np.ascontiguousarray(np.asarray(in_W, dtype=np.float32).astype(ml_dtypes.bfloat16))
    out_w = np.ascontiguousarray(np.asarray(out_W, dtype=np.float32).astype(ml_dtypes.bfloat16))

    in_maps = [
        {"idx_all": idx_l[m], "in_w": in_w, "out_w": out_w}
        for m in range(N_CORES)
    ]
    # Rare per-core HW corruption (can be sticky on a given core) shows up
    # as NaN partials.  Retry with the slice->core assignment ROTATED each
    # attempt so a slice pinned to a bad core is recomputed by a good one.
    vals = np.full(N_CORES, np.nan)
    for rot in range(N_CORES):
        maps = [None] * N_CORES
        for s in range(N_CORES):
            maps[(s + rot) % N_CORES] = in_maps[s]
        res = run_bass_kernel_spmd(nc, maps, core_ids=list(range(N_CORES)))
        for s in range(N_CORES):
            if not np.isfinite(vals[s]):
                part = np.asarray(
                    res.results[(s + rot) % N_CORES]["loss"], dtype=np.float64
                )
                v = part[:, 0].sum() - part[:, 1].sum() / CTX
                if np.isfinite(v):
                    vals[s] = v
        if np.isfinite(vals).all():
            break
    return np.float32(vals.sum() / BATCH)



# revision 2
# speedup vs baseline: 1.1799x; 1.1799x over previous
"""CBOW negative-sampling loss kernel for Trainium2 (8 NeuronCores) — v3.

    loss = mean softplus(-pos) + mean sum_k softplus(neg)
    pos/neg = dots of (sum of 8 in_W ctx rows)/8 with out_W rows.

Data-parallel over batch: 2048 rows/core, tables replicated (bf16,
concatenated into ONE [2V, D] table so each superchunk needs a single
indirect gather of 19 rows per batch row: 8 ctx + center + 10 neg).

The gather is descriptor-rate-bound (~12.5ns/row over 16 SDMA engines
=> ~31us for 38912 rows/core). Everything else hides under it:
  Sync: sem clear + entry release, idx DMA, output DMA
  Pool: 6 gather descriptor-gens, back to back, never blocked
  DVE : ctx fold tree, mul, d-fold tree, segment reduce (2.07us/chunk)
  ACT : softplus = exp -> ln(1+x) with accumulate; raw-pos accumulate

Semaphores are cleared at ENTRY (so the NEFF re-executes cleanly)
behind a one-semaphore release barrier; the exit uses the plain block
barrier after the output DMA completes.
"""

import numpy as np

VOCAB = 100000
DIM = 128
BATCH = 16384
CTX = 8
K_NEG = 10
N_CORES = 8
P = 128

B_CORE = BATCH // N_CORES          # 2048
N_CHUNKS = B_CORE // P             # 16
W_COLS = 1 + K_NEG                 # 11
G_COLS = CTX + W_COLS              # 19 gathered rows per batch row
SC_SIZES = [1, 2, 2, 2, 3, 3, 3]   # chunks per superchunk (sum = 16)
N_SC = len(SC_SIZES)
MAX_SC = max(SC_SIZES)

# Semaphores MUST be cleared at entry: the runtime does not reset them
# between NEFF executions (verified: disabling breaks re-execution).
ENTRY_CLEAR = True
# The runtime quiesces DMA before reading outputs, so the kernel does not
# need to block on the output DMA completion (verified correct; saves ~2.5us).
FINAL_WAIT = False

_CACHE = {}


def build_bass():
    import concourse.bass as bass
    import concourse.mybir as mybir

    f32 = mybir.dt.float32
    bf16 = mybir.dt.bfloat16
    i32 = mybir.dt.int32
    X = mybir.AxisListType.X
    ADD = mybir.AluOpType.add
    MUL = mybir.AluOpType.mult
    AF = mybir.ActivationFunctionType

    assert sum(SC_SIZES) == N_CHUNKS
    starts = np.cumsum([0] + SC_SIZES).tolist()

    nc = bass.Bass(detect_race_conditions=False)

    idx_d = nc.dram_tensor("idx_all", [P, N_CHUNKS * G_COLS], i32,
                           kind="ExternalInput")
    tab_d = nc.dram_tensor("tab", [2 * VOCAB, DIM], bf16, kind="ExternalInput")
    loss_d = nc.dram_tensor("loss", [P, 2 * N_SC], f32, kind="ExternalOutput")

    idx_s = nc.alloc_sbuf_tensor("idx_s", [P, N_CHUNKS * G_COLS], i32)
    g = nc.alloc_sbuf_tensor("g", [P, N_CHUNKS * G_COLS * DIM], bf16)
    t1 = nc.alloc_sbuf_tensor("t1", [P, MAX_SC * 4 * DIM], bf16)
    t2 = nc.alloc_sbuf_tensor("t2", [P, MAX_SC * 2 * DIM], bf16)
    cs = nc.alloc_sbuf_tensor("cs", [P, MAX_SC * DIM], bf16)
    prod = nc.alloc_sbuf_tensor("prod", [P, MAX_SC * W_COLS * DIM], bf16)
    f1 = nc.alloc_sbuf_tensor("f1", [P, MAX_SC * W_COLS * 64], bf16)
    f2 = nc.alloc_sbuf_tensor("f2", [P, MAX_SC * W_COLS * 32], bf16)
    f3 = nc.alloc_sbuf_tensor("f3", [P, MAX_SC * W_COLS * 16], bf16)
    dots = [nc.alloc_sbuf_tensor(f"dots_{i}", [P, MAX_SC * W_COLS], f32) for i in range(2)]
    es = nc.alloc_sbuf_tensor("es", [P, MAX_SC * W_COLS], f32)
    sp = nc.alloc_sbuf_tensor("sp", [P, MAX_SC * W_COLS], f32)
    pos_t = nc.alloc_sbuf_tensor("pos_t", [P, MAX_SC], f32)
    accs = nc.alloc_sbuf_tensor("accs", [P, 2 * N_SC], f32)

    bsem = nc.alloc_semaphore("bsem")             # entry release (sems cleared)
    isem = nc.alloc_semaphore("isem")             # idx load done
    scsems = [nc.alloc_semaphore(f"scsem{s}") for s in range(N_SC)]  # gather s done (>=16)
    dsem = nc.alloc_semaphore("dsem")             # DVE dots_s done (>= s+1)
    asem = nc.alloc_semaphore("asem")             # ACT done with sc s (>= s+1)
    osem = nc.alloc_semaphore("osem")             # output DMA done
    all_sems = [bsem, isem, *scsems, dsem, asem, osem]
    sem_lo = min(s.num for s in all_sems)
    sem_hi = max(s.num for s in all_sems)
    assert sem_hi - sem_lo + 1 == len(all_sems), "sems must be contiguous"

    def sc_view(t, s, per_chunk_elems):
        return t[:, starts[s] * per_chunk_elems:starts[s + 1] * per_chunk_elems]

    c0sem = nc.alloc_semaphore("c0sem")  # sc0 ctx-only gather done
    all_sems.append(c0sem)
    sem_hi = max(sem_hi, c0sem.num)
    assert sem_hi - sem_lo + 1 == len(all_sems), "sems must be contiguous (c0)"

    with nc.Block() as blk:
        @blk.sync
        def _(eng):
            # clear all kernel sems (previous execution left them set),
            # then release the other engines.
            if ENTRY_CLEAR:
                eng.sem_clear(range(sem_lo, sem_hi + 1))
                eng.sem_inc(bsem, 1)
            # idx split: sc0's indices first so desc-gen starts sooner
            eng.dma_start(idx_s[:, :starts[1] * G_COLS],
                          idx_d[:, :starts[1] * G_COLS]).then_inc(isem, 16)
            eng.dma_start(idx_s[:, starts[1] * G_COLS:],
                          idx_d[:, starts[1] * G_COLS:]).then_inc(isem, 16)

        @blk.gpsimd
        def _(eng):
            if ENTRY_CLEAR:
                eng.wait_ge(bsem, 1)
            eng.wait_ge(isem, 16)
            # sc0 splits into ctx-rows and w-rows gathers so the DVE ctx
            # tree starts as early as possible
            n0 = starts[1]
            eng.indirect_dma_start(
                out=g[:, :n0 * CTX * DIM],
                out_offset=None,
                in_=tab_d[:],
                in_offset=bass.IndirectOffsetOnAxis(
                    ap=idx_s[:, :n0 * CTX], axis=0),
            ).then_inc(c0sem, 16)
            eng.indirect_dma_start(
                out=g[:, n0 * CTX * DIM:n0 * G_COLS * DIM],
                out_offset=None,
                in_=tab_d[:],
                in_offset=bass.IndirectOffsetOnAxis(
                    ap=idx_s[:, n0 * CTX:n0 * G_COLS], axis=0),
            ).then_inc(scsems[0], 16)
            eng.wait_ge(isem, 32)
            for s in range(1, N_SC):
                eng.indirect_dma_start(
                    out=sc_view(g, s, G_COLS * DIM),
                    out_offset=None,
                    in_=tab_d[:],
                    in_offset=bass.IndirectOffsetOnAxis(
                        ap=sc_view(idx_s, s, G_COLS), axis=0),
                ).then_inc(scsems[s], 16)

        @blk.vector
        def _(eng):
            if ENTRY_CLEAR:
                eng.wait_ge(bsem, 1)
            for s in range(N_SC):
                sc = SC_SIZES[s]
                if s == 0:
                    eng.wait_ge(c0sem, 16)
                    n0 = starts[1]
                    cv = g[:, :n0 * CTX * DIM].rearrange(
                        "p (c j d) -> p c j d", j=CTX, d=DIM)
                    wv = g[:, n0 * CTX * DIM:n0 * G_COLS * DIM].rearrange(
                        "p (c j d) -> p c j d", j=W_COLS, d=DIM)
                else:
                    eng.wait_ge(scsems[s], 16)
                    gv = sc_view(g, s, G_COLS * DIM).rearrange(
                        "p (c j d) -> p c j d", j=G_COLS, d=DIM)
                    cv = gv[:, :, 0:CTX, :]
                    wv = gv[:, :, CTX:G_COLS, :]
                if s >= 2:
                    eng.wait_ge(asem, s - 1)   # dots buffer parity free
                t1v = t1[:, :sc * 4 * DIM].rearrange(
                    "p (c k d) -> p c k d", k=4, d=DIM)
                eng.tensor_tensor(
                    out=t1v, in0=cv[:, :, 0:4, :], in1=cv[:, :, 4:8, :], op=ADD)
                t2v = t2[:, :sc * 2 * DIM].rearrange(
                    "p (c k d) -> p c k d", k=2, d=DIM)
                eng.tensor_tensor(
                    out=t2v, in0=t1v[:, :, 0:2, :], in1=t1v[:, :, 2:4, :], op=ADD)
                csv = cs[:, :sc * DIM].rearrange("p (c o d) -> p c o d", o=1, d=DIM)
                eng.tensor_tensor(
                    out=csv, in0=t2v[:, :, 0:1, :], in1=t2v[:, :, 1:2, :], op=ADD)
                if s == 0:
                    eng.wait_ge(scsems[0], 16)
                pv = prod[:, :sc * W_COLS * DIM].rearrange(
                    "p (c t d) -> p c t d", t=W_COLS, d=DIM)
                eng.tensor_tensor(
                    out=pv, in0=wv,
                    in1=csv.broadcast_to([P, sc, W_COLS, DIM]), op=MUL)
                pvh = prod[:, :sc * W_COLS * DIM].rearrange(
                    "p (c t h d) -> p c t h d", t=W_COLS, h=2, d=64)
                f1v = f1[:, :sc * W_COLS * 64].rearrange(
                    "p (c t d) -> p c t d", t=W_COLS, d=64)
                eng.tensor_tensor(
                    out=f1v, in0=pvh[:, :, :, 0, :], in1=pvh[:, :, :, 1, :], op=ADD)
                f1h = f1[:, :sc * W_COLS * 64].rearrange(
                    "p (c t h d) -> p c t h d", t=W_COLS, h=2, d=32)
                f2v = f2[:, :sc * W_COLS * 32].rearrange(
                    "p (c t d) -> p c t d", t=W_COLS, d=32)
                eng.tensor_tensor(
                    out=f2v, in0=f1h[:, :, :, 0, :], in1=f1h[:, :, :, 1, :], op=ADD)
                f2h = f2[:, :sc * W_COLS * 32].rearrange(
                    "p (c t h d) -> p c t h d", t=W_COLS, h=2, d=16)
                f3v = f3[:, :sc * W_COLS * 16].rearrange(
                    "p (c t d) -> p c t d", t=W_COLS, d=16)
                eng.tensor_tensor(
                    out=f3v, in0=f2h[:, :, :, 0, :], in1=f2h[:, :, :, 1, :], op=ADD)
                eng.tensor_reduce(
                    out=dots[s % 2][:, :sc * W_COLS],
                    in_=f3v, axis=X, op=ADD,
                ).then_inc(dsem, 1)

        @blk.scalar
        def _(eng):
            if ENTRY_CLEAR:
                eng.wait_ge(bsem, 1)
            for s in range(N_SC):
                sc = SC_SIZES[s]
                eng.wait_ge(dsem, s + 1)
                dv = dots[s % 2][:, :sc * W_COLS]
                eng.activation(out=es[:, :sc * W_COLS], in_=dv,
                               func=AF.Exp, scale=1.0 / CTX)
                eng.activation(out=sp[:, :sc * W_COLS], in_=es[:, :sc * W_COLS],
                               func=AF.Ln, bias=1.0,
                               accum_out=accs[:, s:s + 1])
                eng.activation(
                    out=pos_t[:, :sc].rearrange("p (c o) -> p c o", o=1, c=sc),
                    in_=dots[s % 2][:, :sc * W_COLS].rearrange(
                        "p (c t) -> p c t", t=W_COLS, c=sc)[:, :, 0:1],
                    func=AF.Copy,
                    accum_out=accs[:, N_SC + s:N_SC + s + 1],
                ).then_inc(asem, 1)

        @blk.sync
        def _(eng):
            eng.wait_ge(asem, N_SC)
            eng.dma_start(loss_d[:], accs[:]).then_inc(osem, 16)
            if FINAL_WAIT:
                eng.wait_ge(osem, 16)

    nc.finalize()
    return nc


def pack_indices(center, context, neg_context):
    """idx [P, 16*19]: [p, c*19+j] = context[c*128+p, j] for j<8,
    else VOCAB + (center|neg)[c*128+p, j-8]."""
    rows = N_CHUNKS * P
    out = []
    for m in range(N_CORES):
        lo = m * rows
        ctx = np.asarray(context[lo:lo + rows]).astype(np.int32)
        cen = np.asarray(center[lo:lo + rows]).astype(np.int32)
        neg = np.asarray(neg_context[lo:lo + rows]).astype(np.int32)
        w = np.concatenate([cen.reshape(rows, 1), neg.reshape(rows, K_NEG)],
                           axis=1) + VOCAB
        allidx = np.concatenate([ctx, w], axis=1)          # [rows, 19]
        p = allidx.reshape(N_CHUNKS, P, G_COLS).transpose(1, 0, 2).reshape(
            P, N_CHUNKS * G_COLS)
        out.append(np.ascontiguousarray(p))
    return out


def host_combine(part):
    part = np.asarray(part, dtype=np.float64)
    return part[:, :N_SC].sum() - part[:, N_SC:].sum() / CTX


def kernel(center, context, neg_context, in_W, out_W):
    from concourse.bass_utils import run_bass_kernel_spmd
    import ml_dtypes

    if "nc" not in _CACHE:
        _CACHE["nc"] = build_bass()
    nc = _CACHE["nc"]

    idx_l = pack_indices(center, context, neg_context)
    tab = np.ascontiguousarray(np.concatenate([
        np.asarray(in_W, dtype=np.float32),
        np.asarray(out_W, dtype=np.float32),
    ]).astype(ml_dtypes.bfloat16))

    in_maps = [{"idx_all": idx_l[m], "tab": tab} for m in range(N_CORES)]
    vals = np.full(N_CORES, np.nan)
    for rot in range(N_CORES):
        maps = [None] * N_CORES
        for s in range(N_CORES):
            maps[(s + rot) % N_CORES] = in_maps[s]
        res = run_bass_kernel_spmd(nc, maps, core_ids=list(range(N_CORES)))
        for s in range(N_CORES):
            if not np.isfinite(vals[s]):
                v = host_combine(res.results[(s + rot) % N_CORES]["loss"])
                if np.isfinite(v):
                    vals[s] = v
        if np.isfinite(vals).all():
            break
    return np.float32(vals.sum() / BATCH)
